# revision 18
# baseline (speedup 1.0000x reference)
"""Trainium2 Bass kernel for DeepseekV4 HCA compressor (single-shot window compression).

Computation per 128-token window:
    kv   = h @ w_kv            [128, 128]
    gate = h @ w_gate + bias   [128, 128]
    w    = softmax(gate, axis=tokens)   (per output channel)
    comp = sum(w * kv, axis=tokens)     [128]
then RMS-norm over channels and interleaved RoPE on the last 64 channels.

Sharding: 128 windows (2 batches x 64) split across 8 cores, 16 windows each.

v2 pipeline (per core, per window):
  - DMA h [128 tok, 2048] fp32 (SP ring; ~2.9us/window = the 360 GB/s floor)
  - GPSIMD (idle otherwise) converts fp32 -> bf16 in 4 chunks
  - PE transposes the bf16 chunks (bf16 streams at 1 cycle/row vs fp32's 2;
    the BIR verifier forbids mixing 32-bit and 16-bit matmul operands, so the
    conversion pass is what buys the fast transpose)
  - ACT/DVE copy PSUM->SBUF to build hT (bf16 TensorCopy hits the DVE 2x mode)
  - bf16 matmuls (1 cycle/row at any moving size) accumulate kv/gate per window
  - position bias added into the gate PSUM via an identity matmul
  - ACT exp with fused per-window sum; DVE mul+reduce for the numerator
Per group of 4 windows: transpose comp columns into rows, RMS-norm via a
Newton rsqrt on DVE (no ACT table switches), RoPE, and an early 2KB output
DMA so the tail only carries the last group's epilogue.
"""

import sys

if "/opt/trn_rl_repo" not in sys.path:
    sys.path.insert(0, "/opt/trn_rl_repo")

import ml_dtypes
import numpy as np

import concourse.bacc as bacc
import concourse.mybir as mybir
import concourse.tile as tile
from concourse.bass_utils import run_bass_kernel_spmd
from concourse.masks import make_identity

# Problem shapes (hardcoded per contest contract)
B, S, H = 2, 8192, 2048
M = 128          # compress rate (window length)
D = 128          # head dim
T = S // M       # 64 windows per batch
NCORES = 8
WPC = (B * T) // NCORES   # 16 windows per core
GW = 4                    # windows per output group
GROUPS = WPC // GW        # 4
KC = H // 128             # 16 contraction chunks
KB = KC // 4              # 4 chunk-banks (4 chunks share a PSUM bank)
ROPE_DIM = 64
HALF = ROPE_DIM // 2
THETA = 10000.0
EPS = 1e-6

F32 = mybir.dt.float32
BF16 = mybir.dt.bfloat16
I32 = mybir.dt.int32
AF = mybir.ActivationFunctionType
ALU = mybir.AluOpType


def _build_nc():
    nc = bacc.Bacc(None, target_bir_lowering=False)

    h_in = nc.dram_tensor("h_in", [WPC * M, H], F32, kind="ExternalInput")
    # weights host-preshuffled to [128, KC*D] so the DMA is a straight
    # 4KB-per-partition copy (keeps descriptors >= 512B at bf16)
    wkv_in = nc.dram_tensor("wkv_in", [128, KC * D], BF16, kind="ExternalInput")
    wg_in = nc.dram_tensor("wg_in", [128, KC * D], BF16, kind="ExternalInput")
    bias_in = nc.dram_tensor("bias_in", [D, GW * M], BF16, kind="ExternalInput")
    cs_in = nc.dram_tensor("cs_in", [GW, GROUPS * 2 * ROPE_DIM], F32, kind="ExternalInput")
    wn_in = nc.dram_tensor("wn_in", [GW, D], F32, kind="ExternalInput")
    out_d = nc.dram_tensor("out_d", [WPC, D], F32, kind="ExternalOutput")

    with tile.TileContext(nc) as tc:
        with (
            tc.tile_pool(name="constp", bufs=1) as constp,
            tc.tile_pool(name="hnatp", bufs=6) as hnatp,
            tc.tile_pool(name="hbp", bufs=3) as hbp,
            tc.tile_pool(name="hTp", bufs=2) as hTp,
            tc.tile_pool(name="sfxp", bufs=2) as sfxp,
            tc.tile_pool(name="smallp", bufs=2) as smallp,
            tc.tile_pool(name="grpp", bufs=2) as grpp,
            tc.tile_pool(name="tpp", bufs=3, space="PSUM") as tpp,
            tc.tile_pool(name="mmp", bufs=2, space="PSUM") as mmp,
            tc.tile_pool(name="ctp", bufs=1, space="PSUM") as ctp,
            tc.tile_pool(name="finalp", bufs=1) as finalp,
        ):
            # --- constants ---
            identb = constp.tile([128, 128], BF16, name="identb")
            make_identity(nc, identb)
            identf = constp.tile([128, 128], F32, name="identf")
            make_identity(nc, identf)
            zc = constp.tile([128, 1], F32, name="zc")
            nc.vector.memset(zc[:, :], 0.0)
            # preload the exp ACT table while the first DMAs run
            warm = constp.tile([128, 1], F32, name="warm")
            nc.scalar.activation(warm[:, :], zc[:, :], AF.Exp, bias=zc[:, :])

            # small constants ride the ACT ring so the SP ring streams h
            cs_sb = constp.tile([GW, GROUPS * 2 * ROPE_DIM], F32, name="cs_sb")
            nc.scalar.dma_start(out=cs_sb, in_=cs_in[:, :])
            wn_sb = constp.tile([GW, D], F32, name="wn_sb")
            nc.scalar.dma_start(out=wn_sb, in_=wn_in[:, :])

            wkv_sb = constp.tile([128, KC * D], BF16, name="wkv_sb")
            wg_sb = constp.tile([128, KC * D], BF16, name="wg_sb")
            bias_sb = constp.tile([D, GW * M], BF16, name="bias_sb")

            comp = finalp.tile([D, WPC], F32, name="comp")

            GM = GW * M  # 512: group moving dim

            for g in range(GROUPS):
                last_group = g == GROUPS - 1
                hT = hTp.tile([128, KC * GM], BF16, name="hT", tag="hT")
                if last_group:
                    # per-window PSUM tiles reuse the group tags (same banks,
                    # sequential accumulation groups — never interleaved)
                    gt_w = [mmp.tile([D, M], F32, name="gt_w", tag="gt") for _ in range(GW)]
                    kv_w = [mmp.tile([D, M], F32, name="kv_w", tag="kv") for _ in range(GW)]
                for i in range(GW):
                    w = g * GW + i
                    hnat = hnatp.tile([128, H], F32, name="hnat", tag="hnat")
                    tok0 = w * M
                    if w == 0 or w == WPC - 1:
                        # finer chunks: first window so transposes start
                        # earlier, last window to shorten the tail
                        for kb in range(KB):
                            nc.sync.dma_start(
                                out=hnat[:, kb * 512 : (kb + 1) * 512],
                                in_=h_in[tok0 : tok0 + M, kb * 512 : (kb + 1) * 512],
                            )
                    else:
                        nc.sync.dma_start(out=hnat, in_=h_in[tok0 : tok0 + M, :])
                    if w == 0:
                        # weights land behind window 0 on the same SP ring: PE
                        # stalls ~2us on the first group's matmuls but recovers
                        nc.sync.dma_start(out=wkv_sb, in_=wkv_in[:, :])
                        nc.sync.dma_start(out=wg_sb, in_=wg_in[:, :])
                        nc.sync.dma_start(out=bias_sb, in_=bias_in[:, :])

                    hb = hbp.tile([128, H], BF16, name="hb", tag="hb")
                    hTk = hT.rearrange("p (k t) -> p k t", k=KC)
                    for kb in range(KB):
                        # fp32->bf16 conversion: GPSIMD runs at 0.6 efficiency
                        # (806ns/chunk), so 3 chunks keep Pool under the 2.9us
                        # DMA cadence and ACT takes the 4th
                        src = hnat[:, kb * 512 : (kb + 1) * 512]
                        cdst = hb[:, kb * 512 : (kb + 1) * 512]
                        if kb < 3:
                            nc.gpsimd.tensor_copy(cdst, src)
                        else:
                            nc.scalar.copy(cdst, src)
                        tp = tpp.tile([128, 512], BF16, name="tp", tag="tp")
                        for j in range(4):
                            k = kb * 4 + j
                            nc.tensor.matmul(
                                tp[:, j * M : (j + 1) * M],
                                hb[:, k * 128 : (k + 1) * 128],
                                identb[:, :],
                                is_transpose=True,
                                start=(j == 0),
                                stop=(j == 3),
                            )
                        # scatter the 4 chunks into hT at window slot i
                        dst = hTk[:, kb * 4 : (kb + 1) * 4, i * M : (i + 1) * M]
                        srcr = tp.rearrange("p (c m) -> p c m", c=4)
                        # bf16 TensorCopy hits the DVE 2x mode — DVE takes 3,
                        # ACT takes 1
                        if kb < 3:
                            nc.vector.tensor_copy(dst, srcr)
                        else:
                            nc.scalar.copy(dst, srcr)

                    if last_group:
                        # per-window matmuls + softmax: short serial tail
                        # after the final DMA byte
                        gt_ps, kv_ps = gt_w[i], kv_w[i]
                        for k in range(KC):
                            nc.tensor.matmul(
                                gt_ps[:, :],
                                wg_sb[:, k * D : (k + 1) * D],
                                hTk[:, k, i * M : (i + 1) * M],
                                start=(k == 0),
                                stop=False,
                            )
                            nc.tensor.matmul(
                                kv_ps[:, :],
                                wkv_sb[:, k * D : (k + 1) * D],
                                hTk[:, k, i * M : (i + 1) * M],
                                start=(k == 0),
                                stop=(k == KC - 1),
                            )
                        nc.tensor.matmul(
                            gt_ps[:, :], identb[:, :], bias_sb[:, 0:M],
                            start=False, stop=True,
                        )
                        e_sb = sfxp.tile([D, M], F32, name="e_sb", tag="e")
                        prod = sfxp.tile([D, M], F32, name="prod", tag="prod")
                        den = smallp.tile([D, 1], F32, name="den", tag="den")
                        num = smallp.tile([D, 1], F32, name="num", tag="num")
                        rden = smallp.tile([D, 1], F32, name="rden", tag="rden")
                        nc.scalar.activation(
                            e_sb[:, :], gt_ps[:, :], AF.Exp, bias=zc[:D, :],
                            accum_out=den[:, :],
                        )
                        nc.vector.tensor_mul(prod[:, :], e_sb[:, :], kv_ps[:, :])
                        nc.vector.tensor_reduce(
                            num[:, :], prod[:, :], axis=mybir.AxisListType.X,
                            op=ALU.add,
                        )
                        nc.vector.reciprocal(rden[:, :], den[:, :])
                        nc.vector.tensor_mul(
                            comp[:, w : w + 1], num[:, :], rden[:, :]
                        )

                if not last_group:
                    # group-level projections: moving dim 512 quarters the PE
                    # instruction count (PE SEQ issue is a real limit), same
                    # engine cycles. kv chunks 0-11 are emitted before gate
                    # chunks 12-15 so PE has ready work while the last
                    # window's final copies drain.
                    gt_ps = mmp.tile([D, GM], F32, name="gt_ps", tag="gt")
                    kv_ps = mmp.tile([D, GM], F32, name="kv_ps", tag="kv")
                    for k in range(12):
                        nc.tensor.matmul(
                            gt_ps[:, :],
                            wg_sb[:, k * D : (k + 1) * D],
                            hT[:, k * GM : (k + 1) * GM],
                            start=(k == 0),
                            stop=False,
                        )
                    for k in range(12):
                        nc.tensor.matmul(
                            kv_ps[:, :],
                            wkv_sb[:, k * D : (k + 1) * D],
                            hT[:, k * GM : (k + 1) * GM],
                            start=(k == 0),
                            stop=False,
                        )
                    for k in range(12, KC):
                        nc.tensor.matmul(
                            gt_ps[:, :],
                            wg_sb[:, k * D : (k + 1) * D],
                            hT[:, k * GM : (k + 1) * GM],
                            start=False,
                            stop=False,
                        )
                    nc.tensor.matmul(
                        gt_ps[:, :], identb[:, :], bias_sb[:, :],
                        start=False, stop=True,
                    )
                    for k in range(12, KC):
                        nc.tensor.matmul(
                            kv_ps[:, :],
                            wkv_sb[:, k * D : (k + 1) * D],
                            hT[:, k * GM : (k + 1) * GM],
                            start=False,
                            stop=(k == KC - 1),
                        )

                    # group softmax: one exp, per-window reduces
                    e4 = sfxp.tile([D, GM], F32, name="e4", tag="e")
                    prod4 = sfxp.tile([D, GM], F32, name="prod4", tag="prod")
                    den4 = smallp.tile([D, GW], F32, name="den4", tag="den")
                    num4 = smallp.tile([D, GW], F32, name="num4", tag="num")
                    rden4 = smallp.tile([D, GW], F32, name="rden4", tag="rden")
                    nc.scalar.activation(
                        e4[:, :], gt_ps[:, :], AF.Exp, bias=zc[:D, :]
                    )
                    nc.vector.tensor_reduce(
                        den4[:, :],
                        e4.rearrange("p (w m) -> p w m", w=GW),
                        axis=mybir.AxisListType.X,
                        op=ALU.add,
                    )
                    nc.vector.tensor_mul(prod4[:, :], e4[:, :], kv_ps[:, :])
                    nc.vector.tensor_reduce(
                        num4[:, :],
                        prod4.rearrange("p (w m) -> p w m", w=GW),
                        axis=mybir.AxisListType.X,
                        op=ALU.add,
                    )
                    nc.vector.reciprocal(rden4[:, :], den4[:, :])
                    nc.vector.tensor_mul(
                        comp[:, g * GW : (g + 1) * GW], num4[:, :], rden4[:, :]
                    )

                # --- per-group epilogue (groups 0..2 fully overlap the
                # stream; only the last group's chain is in the tail) ---
                ct4 = ctp.tile([GW, D], F32, name="ct4", tag="ct4")
                nc.tensor.transpose(
                    ct4[:, :], comp[:, g * GW : (g + 1) * GW], identf[:, :]
                )
                ctg = grpp.tile([GW, D], F32, name="ctg", tag="ctg")
                nc.scalar.copy(ctg[:, :], ct4[:, :])
                sqs = grpp.tile([GW, D], F32, name="sqs", tag="sqs")
                ssq = grpp.tile([GW, 1], F32, name="ssq", tag="ssq")
                nc.scalar.activation(
                    sqs[:, :], ctg[:, :], AF.Square, bias=zc[:GW, :],
                    accum_out=ssq[:, :],
                )
                # norm weight + RoPE on unscaled rows first (rinv is per-row,
                # so it commutes with the elementwise rope); DVE does this
                # while ACT computes the square-sum
                t0 = grpp.tile([GW, D], F32, name="t0", tag="t0")
                nc.vector.tensor_mul(t0[:, :], ctg[:, :], wn_sb[:, :])
                t1 = grpp.tile([GW, ROPE_DIM], F32, name="t1", tag="t1")
                t2 = grpp.tile([GW, ROPE_DIM], F32, name="t2", tag="t2")
                c0 = g * 2 * ROPE_DIM
                nc.vector.tensor_mul(
                    t1[:, :], t0[:, D - ROPE_DIM : D], cs_sb[:, c0 : c0 + ROPE_DIM]
                )
                nc.vector.tensor_mul(
                    t2[:, 0:HALF],
                    t0[:, D - HALF : D],
                    cs_sb[:, c0 + ROPE_DIM : c0 + ROPE_DIM + HALF],
                )
                nc.vector.tensor_mul(
                    t2[:, HALF:ROPE_DIM],
                    t0[:, D - ROPE_DIM : D - HALF],
                    cs_sb[:, c0 + ROPE_DIM + HALF : c0 + 2 * ROPE_DIM],
                )
                nc.vector.tensor_add(t0[:, D - ROPE_DIM : D], t1[:, :], t2[:, :])
                # rinv = 1/sqrt(ssq/D + eps): bit-trick + 1 Newton step on DVE
                # (~1.7e-3 rel err, well inside budget; no ACT table switches)
                vv = grpp.tile([GW, 1], F32, name="vv", tag="vv")
                nc.vector.tensor_scalar(
                    out=vv[:, :], in0=ssq[:, :], scalar1=1.0 / D, scalar2=EPS,
                    op0=ALU.mult, op1=ALU.add,
                )
                rinv = grpp.tile([GW, 1], F32, name="rinv", tag="rinv")
                nc.vector.tensor_scalar(
                    out=rinv.bitcast(I32), in0=vv.bitcast(I32),
                    scalar1=1, scalar2=None, op0=ALU.arith_shift_right,
                )
                nc.vector.tensor_scalar(
                    out=rinv.bitcast(I32), in0=rinv.bitcast(I32),
                    scalar1=-1, scalar2=None, op0=ALU.bitwise_xor,
                )
                nc.vector.tensor_scalar(
                    out=rinv.bitcast(I32), in0=rinv.bitcast(I32),
                    scalar1=0x5F3759DF + 1, scalar2=None, op0=ALU.add,
                )
                nt = grpp.tile([GW, 1], F32, name="nt", tag="nt")
                nc.vector.tensor_mul(nt[:, :], rinv[:, :], rinv[:, :])
                nc.vector.tensor_mul(nt[:, :], nt[:, :], vv[:, :])
                nc.vector.tensor_scalar(
                    out=nt[:, :], in0=nt[:, :], scalar1=-0.5, scalar2=1.5,
                    op0=ALU.mult, op1=ALU.add,
                )
                nc.vector.tensor_mul(rinv[:, :], rinv[:, :], nt[:, :])
                outg = grpp.tile([GW, D], F32, name="outg", tag="outg")
                nc.vector.tensor_scalar_mul(outg[:, :], t0[:, :], rinv[:, :])
                nc.sync.dma_start(
                    out=out_d[g * GW : (g + 1) * GW, :], in_=outg[:, :]
                )

    nc.compile()
    return nc


_NC_CACHE = {}


def _get_nc():
    if "nc" not in _NC_CACHE:
        _NC_CACHE["nc"] = _build_nc()
    return _NC_CACHE["nc"]


def _make_in_maps(hidden_states, w_kv, w_gate, position_bias, kv_norm_weight):
    hidden_states = np.ascontiguousarray(np.asarray(hidden_states, dtype=np.float32))
    w_kv = np.asarray(w_kv, dtype=np.float32)
    w_gate = np.asarray(w_gate, dtype=np.float32)
    position_bias = np.asarray(position_bias, dtype=np.float32)
    kv_norm_weight = np.asarray(kv_norm_weight, dtype=np.float32)

    h_flat = hidden_states.reshape(B * S, H)
    # [H, D] -> [128, KC*D] so chunk k sits at cols [k*D, (k+1)*D)
    def shuffle_w(w):
        return np.ascontiguousarray(
            w.reshape(KC, 128, D).transpose(1, 0, 2).reshape(128, KC * D)
        ).astype(ml_dtypes.bfloat16)

    wkv_h = shuffle_w(w_kv)
    wg_h = shuffle_w(w_gate)
    bias_h = np.ascontiguousarray(np.tile(position_bias.T, (1, GW))).astype(
        ml_dtypes.bfloat16
    )
    wn = np.ascontiguousarray(np.broadcast_to(kv_norm_weight[None, :], (GW, D)))

    inv_freq = (1.0 / (THETA ** (np.arange(HALF, dtype=np.float32) / HALF))).astype(
        np.float32
    )
    in_maps = []
    for c in range(NCORES):
        t_global = (c % (T // WPC)) * WPC + np.arange(WPC, dtype=np.float32)
        pos = (t_global * M).astype(np.float32)
        freqs = pos[:, None] * inv_freq[None, :]
        cos2 = np.repeat(np.cos(freqs), 2, axis=1).astype(np.float32)
        sin2 = np.repeat(np.sin(freqs), 2, axis=1).astype(np.float32)
        sinf = np.concatenate([-sin2[:, :HALF], sin2[:, HALF:]], axis=1)
        cs16 = np.concatenate([cos2, sinf], axis=1)  # [16, 128]
        # window w = 4g+i -> cs[i, g*128 : (g+1)*128]
        cs = np.zeros((GW, GROUPS * 2 * ROPE_DIM), np.float32)
        for g in range(GROUPS):
            for i in range(GW):
                cs[i, g * 128 : (g + 1) * 128] = cs16[g * GW + i]
        cs = np.ascontiguousarray(cs)
        in_maps.append(
            {
                "h_in": h_flat[c * WPC * M : (c + 1) * WPC * M],
                "wkv_in": wkv_h,
                "wg_in": wg_h,
                "bias_in": bias_h,
                "cs_in": cs,
                "wn_in": wn,
            }
        )
    return in_maps


def _assemble(results):
    full = np.concatenate([r["out_d"] for r in results], axis=0)  # [128, 128]
    return full.reshape(B, 1, T, D).astype(np.float32)


def _run(inputs, trace=False, **spmd_kwargs):
    nc = _get_nc()
    in_maps = _make_in_maps(
        inputs["hidden_states"],
        inputs["w_kv"],
        inputs["w_gate"],
        inputs["position_bias"],
        inputs["kv_norm_weight"],
    )
    res = run_bass_kernel_spmd(
        nc, in_maps, core_ids=list(range(NCORES)), trace=trace, **spmd_kwargs
    )
    return _assemble(res.results), res


def kernel(
    hidden_states,
    q_residual=None,
    position_ids=None,
    w_kv=None,
    w_gate=None,
    position_bias=None,
    kv_norm_weight=None,
):
    out, _ = _run(
        {
            "hidden_states": hidden_states,
            "w_kv": w_kv,
            "w_gate": w_gate,
            "position_bias": position_bias,
            "kv_norm_weight": kv_norm_weight,
        }
    )
    return out


# revision 20
# speedup vs baseline: 1.0984x; 1.0984x over previous
"""Trainium2 Bass kernel for DeepseekV4 HCA compressor (single-shot window compression).

Computation per 128-token window:
    kv   = h @ w_kv            [128, 128]
    gate = h @ w_gate + bias   [128, 128]
    w    = softmax(gate, axis=tokens)   (per output channel)
    comp = sum(w * kv, axis=tokens)     [128]
then RMS-norm over channels and interleaved RoPE on the last 64 channels.

Sharding: 128 windows (2 batches x 64) split across 8 cores, 16 windows each.

v2 pipeline (per core, per window):
  - DMA h [128 tok, 2048] fp32 (SP ring; ~2.9us/window = the 360 GB/s floor)
  - GPSIMD (idle otherwise) converts fp32 -> bf16 in 4 chunks
  - PE transposes the bf16 chunks (bf16 streams at 1 cycle/row vs fp32's 2;
    the BIR verifier forbids mixing 32-bit and 16-bit matmul operands, so the
    conversion pass is what buys the fast transpose)
  - ACT/DVE copy PSUM->SBUF to build hT (bf16 TensorCopy hits the DVE 2x mode)
  - bf16 matmuls (1 cycle/row at any moving size) accumulate kv/gate per window
  - position bias added into the gate PSUM via an identity matmul
  - ACT exp with fused per-window sum; DVE mul+reduce for the numerator
Per group of 4 windows: transpose comp columns into rows, RMS-norm via a
Newton rsqrt on DVE (no ACT table switches), RoPE, and an early 2KB output
DMA so the tail only carries the last group's epilogue.
"""

import sys

if "/opt/trn_rl_repo" not in sys.path:
    sys.path.insert(0, "/opt/trn_rl_repo")

import ml_dtypes
import numpy as np

import concourse.bacc as bacc
import concourse.mybir as mybir
import concourse.tile as tile
from concourse.bass_utils import run_bass_kernel_spmd
from concourse.masks import make_identity

# Problem shapes (hardcoded per contest contract)
B, S, H = 2, 8192, 2048
M = 128          # compress rate (window length)
D = 128          # head dim
T = S // M       # 64 windows per batch
NCORES = 8
WPC = (B * T) // NCORES   # 16 windows per core
GW = 4                    # windows per output group
GROUPS = WPC // GW        # 4
KC = H // 128             # 16 contraction chunks
KB = KC // 4              # 4 chunk-banks (4 chunks share a PSUM bank)
ROPE_DIM = 64
HALF = ROPE_DIM // 2
THETA = 10000.0
EPS = 1e-6

F32 = mybir.dt.float32
BF16 = mybir.dt.bfloat16
I32 = mybir.dt.int32
AF = mybir.ActivationFunctionType
ALU = mybir.AluOpType


def _build_nc():
    nc = bacc.Bacc(None, target_bir_lowering=False)

    h_in = nc.dram_tensor("h_in", [WPC * M, H], F32, kind="ExternalInput")
    # weights host-preshuffled to [128, KC*D] so the DMA is a straight
    # 4KB-per-partition copy (keeps descriptors >= 512B at bf16)
    wkv_in = nc.dram_tensor("wkv_in", [128, KC * D], BF16, kind="ExternalInput")
    wg_in = nc.dram_tensor("wg_in", [128, KC * D], BF16, kind="ExternalInput")
    bias_in = nc.dram_tensor("bias_in", [D, GW * M], BF16, kind="ExternalInput")
    cs_in = nc.dram_tensor("cs_in", [GW, GROUPS * 2 * ROPE_DIM], F32, kind="ExternalInput")
    wn_in = nc.dram_tensor("wn_in", [GW, D], F32, kind="ExternalInput")
    out_d = nc.dram_tensor("out_d", [WPC, D], F32, kind="ExternalOutput")

    with tile.TileContext(nc) as tc:
        with (
            tc.tile_pool(name="constp", bufs=1) as constp,
            tc.tile_pool(name="hnatp", bufs=6) as hnatp,
            tc.tile_pool(name="hbp", bufs=3) as hbp,
            tc.tile_pool(name="hTp", bufs=2) as hTp,
            tc.tile_pool(name="sfxp", bufs=2) as sfxp,
            tc.tile_pool(name="smallp", bufs=2) as smallp,
            tc.tile_pool(name="grpp", bufs=2) as grpp,
            tc.tile_pool(name="tpp", bufs=3, space="PSUM") as tpp,
            tc.tile_pool(name="mmp", bufs=2, space="PSUM") as mmp,
            tc.tile_pool(name="ctp", bufs=1, space="PSUM") as ctp,
            tc.tile_pool(name="finalp", bufs=1) as finalp,
        ):
            # --- constants ---
            identb = constp.tile([128, 128], BF16, name="identb")
            make_identity(nc, identb)
            identf = constp.tile([128, 128], F32, name="identf")
            make_identity(nc, identf)
            zc = constp.tile([128, 1], F32, name="zc")
            nc.vector.memset(zc[:, :], 0.0)
            # preload the exp ACT table while the first DMAs run
            warm = constp.tile([128, 1], F32, name="warm")
            nc.scalar.activation(warm[:, :], zc[:, :], AF.Exp, bias=zc[:, :])

            # small constants ride the ACT ring so the SP ring streams h
            cs_sb = constp.tile([GW, GROUPS * 2 * ROPE_DIM], F32, name="cs_sb")
            nc.scalar.dma_start(out=cs_sb, in_=cs_in[:, :])
            wn_sb = constp.tile([GW, D], F32, name="wn_sb")
            nc.scalar.dma_start(out=wn_sb, in_=wn_in[:, :])

            wkv_sb = constp.tile([128, KC * D], BF16, name="wkv_sb")
            wg_sb = constp.tile([128, KC * D], BF16, name="wg_sb")
            bias_sb = constp.tile([D, GW * M], BF16, name="bias_sb")

            comp = finalp.tile([D, WPC], F32, name="comp")

            def emit_epilogue(g):
                """Group epilogue: transpose comp columns to rows, RMS-norm
                via Pool-engine Newton rsqrt (keeps DVE/ACT queues free for
                the steady-state pipeline), RoPE, early 2KB output DMA."""
                ct4 = ctp.tile([GW, D], F32, name="ct4", tag="ct4")
                nc.tensor.transpose(
                    ct4[:, :], comp[:, g * GW : (g + 1) * GW], identf[:, :]
                )
                sqs = grpp.tile([GW, D], F32, name="sqs", tag="sqs")
                ssq = grpp.tile([GW, 1], F32, name="ssq", tag="ssq")
                nc.scalar.activation(
                    sqs[:, :], ct4[:, :], AF.Square, bias=zc[:GW, :],
                    accum_out=ssq[:, :],
                )
                # norm weight + RoPE on unscaled rows (rinv is per-row, so it
                # commutes with the elementwise rope); DVE does this while
                # Pool runs the rsqrt chain
                t0 = grpp.tile([GW, D], F32, name="t0", tag="t0")
                nc.vector.tensor_mul(t0[:, :], ct4[:, :], wn_sb[:, :])
                t1 = grpp.tile([GW, ROPE_DIM], F32, name="t1", tag="t1")
                t2 = grpp.tile([GW, ROPE_DIM], F32, name="t2", tag="t2")
                c0 = g * 2 * ROPE_DIM
                nc.vector.tensor_mul(
                    t1[:, :], t0[:, D - ROPE_DIM : D], cs_sb[:, c0 : c0 + ROPE_DIM]
                )
                nc.vector.tensor_mul(
                    t2[:, 0:HALF],
                    t0[:, D - HALF : D],
                    cs_sb[:, c0 + ROPE_DIM : c0 + ROPE_DIM + HALF],
                )
                nc.vector.tensor_mul(
                    t2[:, HALF:ROPE_DIM],
                    t0[:, D - ROPE_DIM : D - HALF],
                    cs_sb[:, c0 + ROPE_DIM + HALF : c0 + 2 * ROPE_DIM],
                )
                nc.vector.tensor_add(t0[:, D - ROPE_DIM : D], t1[:, :], t2[:, :])
                # rinv = 1/sqrt(ssq/D + eps): bit-trick + 1 Newton step
                # (~1.7e-3 rel err; no ACT table switches)
                vv = grpp.tile([GW, 1], F32, name="vv", tag="vv")
                nc.vector.tensor_scalar(
                    out=vv[:, :], in0=ssq[:, :], scalar1=1.0 / D, scalar2=EPS,
                    op0=ALU.mult, op1=ALU.add,
                )
                rinv = grpp.tile([GW, 1], F32, name="rinv", tag="rinv")
                nc.vector.tensor_scalar(
                    out=rinv.bitcast(I32), in0=vv.bitcast(I32),
                    scalar1=1, scalar2=-1,
                    op0=ALU.arith_shift_right, op1=ALU.bitwise_xor,
                )
                nc.vector.tensor_scalar(
                    out=rinv.bitcast(I32), in0=rinv.bitcast(I32),
                    scalar1=0x5F3759DF + 1, scalar2=None, op0=ALU.add,
                )
                nt = grpp.tile([GW, 1], F32, name="nt", tag="nt")
                nc.vector.tensor_mul(nt[:, :], rinv[:, :], rinv[:, :])
                nc.vector.tensor_mul(nt[:, :], nt[:, :], vv[:, :])
                nc.vector.tensor_scalar(
                    out=nt[:, :], in0=nt[:, :], scalar1=-0.5, scalar2=1.5,
                    op0=ALU.mult, op1=ALU.add,
                )
                nc.vector.tensor_mul(rinv[:, :], rinv[:, :], nt[:, :])
                outg = grpp.tile([GW, D], F32, name="outg", tag="outg")
                nc.vector.tensor_scalar_mul(outg[:, :], t0[:, :], rinv[:, :])
                nc.sync.dma_start(
                    out=out_d[g * GW : (g + 1) * GW, :], in_=outg[:, :]
                )

            pending_epi = None
            for w in range(WPC):
                last_w = w == WPC - 1
                hnat = hnatp.tile([128, H], F32, name="hnat", tag="hnat")
                tok0 = w * M
                if w == 0 or last_w:
                    # finer chunks: first window so transposes start earlier,
                    # last window to shorten the tail
                    for kb in range(KB):
                        nc.sync.dma_start(
                            out=hnat[:, kb * 512 : (kb + 1) * 512],
                            in_=h_in[tok0 : tok0 + M, kb * 512 : (kb + 1) * 512],
                        )
                else:
                    nc.sync.dma_start(out=hnat, in_=h_in[tok0 : tok0 + M, :])
                if w == 0:
                    # weights land behind window 0 on the same SP ring: PE
                    # stalls ~2us on the first window's matmuls but recovers
                    nc.sync.dma_start(out=wkv_sb, in_=wkv_in[:, :])
                    nc.sync.dma_start(out=wg_sb, in_=wg_in[:, :])
                    nc.sync.dma_start(out=bias_sb, in_=bias_in[:, :])

                hb = hbp.tile([128, H], BF16, name="hb", tag="hb")
                hT = hTp.tile([128, KC * M], BF16, name="hT", tag="hT")
                for kb in range(KB):
                    src = hnat[:, kb * 512 : (kb + 1) * 512]
                    tp = tpp.tile([128, 512], F32 if last_w else BF16,
                                  name="tp", tag="tp")
                    if last_w:
                        # last window: fp32 transposes straight from hnat
                        # (2 cycles/row but drops the conversion stage from
                        # the tail's serial chain)
                        for j in range(4):
                            k = kb * 4 + j
                            nc.tensor.matmul(
                                tp[:, j * M : (j + 1) * M],
                                hnat[:, k * 128 : (k + 1) * 128],
                                identf[:, :],
                                is_transpose=True,
                                start=(j == 0),
                                stop=(j == 3),
                            )
                    else:
                        # fp32->bf16 conversion: GPSIMD (0.6 efficiency,
                        # 806ns/chunk) takes 3 chunks, ACT the 4th; bf16
                        # transposes then stream at 1 cycle/row vs fp32's 2
                        cdst = hb[:, kb * 512 : (kb + 1) * 512]
                        if kb < 3:
                            nc.gpsimd.tensor_copy(cdst, src)
                        else:
                            nc.scalar.copy(cdst, src)
                        for j in range(4):
                            k = kb * 4 + j
                            nc.tensor.matmul(
                                tp[:, j * M : (j + 1) * M],
                                hb[:, k * 128 : (k + 1) * 128],
                                identb[:, :],
                                is_transpose=True,
                                start=(j == 0),
                                stop=(j == 3),
                            )
                    dst = hT[:, kb * 512 : (kb + 1) * 512]
                    # bf16 TensorCopy hits the DVE 2x mode - DVE takes 3,
                    # ACT takes 1
                    if kb < 3:
                        nc.vector.tensor_copy(dst, tp[:, :])
                    else:
                        nc.scalar.copy(dst, tp[:, :])

                # one-window-delayed epilogue emission: its small ops queue
                # BEHIND this window's copies so they never head-block the
                # steady-state pipeline
                if pending_epi is not None:
                    emit_epilogue(pending_epi)
                    pending_epi = None

                # per-window projections: bf16 runs 1 cycle/row at moving=128.
                # gate and kv need separate PSUM tiles (interleaved
                # accumulation groups in one tile corrupt the first group).
                gt_ps = mmp.tile([D, M], F32, name="gt_ps", tag="gt")
                kv_ps = mmp.tile([D, M], F32, name="kv_ps", tag="kv")
                for kb in range(KB):
                    for j in range(4):
                        k = kb * 4 + j
                        nc.tensor.matmul(
                            gt_ps[:, :],
                            wg_sb[:, k * D : (k + 1) * D],
                            hT[:, k * M : (k + 1) * M],
                            start=(k == 0),
                            stop=False,
                        )
                    for j in range(4):
                        k = kb * 4 + j
                        nc.tensor.matmul(
                            kv_ps[:, :],
                            wkv_sb[:, k * D : (k + 1) * D],
                            hT[:, k * M : (k + 1) * M],
                            start=(k == 0),
                            stop=(k == KC - 1),
                        )
                # gate += position_bias via identity matmul (broadcast-free)
                nc.tensor.matmul(
                    gt_ps[:, :], identb[:, :], bias_sb[:, 0:M],
                    start=False, stop=True,
                )

                # softmax-weighted reduction over the window's tokens
                e_sb = sfxp.tile([D, M], F32, name="e_sb", tag="e")
                prod = sfxp.tile([D, M], F32, name="prod", tag="prod")
                den = smallp.tile([D, 1], F32, name="den", tag="den")
                num = smallp.tile([D, 1], F32, name="num", tag="num")
                rden = smallp.tile([D, 1], F32, name="rden", tag="rden")
                nc.scalar.activation(
                    e_sb[:, :], gt_ps[:, :], AF.Exp, bias=zc[:D, :],
                    accum_out=den[:, :],
                )
                nc.vector.tensor_mul(prod[:, :], e_sb[:, :], kv_ps[:, :])
                nc.vector.tensor_reduce(
                    num[:, :], prod[:, :], axis=mybir.AxisListType.X, op=ALU.add
                )
                nc.vector.reciprocal(rden[:, :], den[:, :])
                nc.vector.tensor_mul(comp[:, w : w + 1], num[:, :], rden[:, :])

                if w % GW == GW - 1:
                    if last_w:
                        emit_epilogue(w // GW)
                    else:
                        pending_epi = w // GW

    nc.compile()
    return nc


_NC_CACHE = {}


def _get_nc():
    if "nc" not in _NC_CACHE:
        _NC_CACHE["nc"] = _build_nc()
    return _NC_CACHE["nc"]


def _make_in_maps(hidden_states, w_kv, w_gate, position_bias, kv_norm_weight):
    hidden_states = np.ascontiguousarray(np.asarray(hidden_states, dtype=np.float32))
    w_kv = np.asarray(w_kv, dtype=np.float32)
    w_gate = np.asarray(w_gate, dtype=np.float32)
    position_bias = np.asarray(position_bias, dtype=np.float32)
    kv_norm_weight = np.asarray(kv_norm_weight, dtype=np.float32)

    h_flat = hidden_states.reshape(B * S, H)
    # [H, D] -> [128, KC*D] so chunk k sits at cols [k*D, (k+1)*D)
    def shuffle_w(w):
        return np.ascontiguousarray(
            w.reshape(KC, 128, D).transpose(1, 0, 2).reshape(128, KC * D)
        ).astype(ml_dtypes.bfloat16)

    wkv_h = shuffle_w(w_kv)
    wg_h = shuffle_w(w_gate)
    bias_h = np.ascontiguousarray(np.tile(position_bias.T, (1, GW))).astype(
        ml_dtypes.bfloat16
    )
    wn = np.ascontiguousarray(np.broadcast_to(kv_norm_weight[None, :], (GW, D)))

    inv_freq = (1.0 / (THETA ** (np.arange(HALF, dtype=np.float32) / HALF))).astype(
        np.float32
    )
    in_maps = []
    for c in range(NCORES):
        t_global = (c % (T // WPC)) * WPC + np.arange(WPC, dtype=np.float32)
        pos = (t_global * M).astype(np.float32)
        freqs = pos[:, None] * inv_freq[None, :]
        cos2 = np.repeat(np.cos(freqs), 2, axis=1).astype(np.float32)
        sin2 = np.repeat(np.sin(freqs), 2, axis=1).astype(np.float32)
        sinf = np.concatenate([-sin2[:, :HALF], sin2[:, HALF:]], axis=1)
        cs16 = np.concatenate([cos2, sinf], axis=1)  # [16, 128]
        # window w = 4g+i -> cs[i, g*128 : (g+1)*128]
        cs = np.zeros((GW, GROUPS * 2 * ROPE_DIM), np.float32)
        for g in range(GROUPS):
            for i in range(GW):
                cs[i, g * 128 : (g + 1) * 128] = cs16[g * GW + i]
        cs = np.ascontiguousarray(cs)
        in_maps.append(
            {
                "h_in": h_flat[c * WPC * M : (c + 1) * WPC * M],
                "wkv_in": wkv_h,
                "wg_in": wg_h,
                "bias_in": bias_h,
                "cs_in": cs,
                "wn_in": wn,
            }
        )
    return in_maps


def _assemble(results):
    full = np.concatenate([r["out_d"] for r in results], axis=0)  # [128, 128]
    return full.reshape(B, 1, T, D).astype(np.float32)


def _run(inputs, trace=False, **spmd_kwargs):
    nc = _get_nc()
    in_maps = _make_in_maps(
        inputs["hidden_states"],
        inputs["w_kv"],
        inputs["w_gate"],
        inputs["position_bias"],
        inputs["kv_norm_weight"],
    )
    res = run_bass_kernel_spmd(
        nc, in_maps, core_ids=list(range(NCORES)), trace=trace, **spmd_kwargs
    )
    return _assemble(res.results), res


def kernel(
    hidden_states,
    q_residual=None,
    position_ids=None,
    w_kv=None,
    w_gate=None,
    position_bias=None,
    kv_norm_weight=None,
):
    out, _ = _run(
        {
            "hidden_states": hidden_states,
            "w_kv": w_kv,
            "w_gate": w_gate,
            "position_bias": position_bias,
            "kv_norm_weight": kv_norm_weight,
        }
    )
    return out


# revision 22
# speedup vs baseline: 1.1124x; 1.0128x over previous
"""Trainium2 Bass kernel for DeepseekV4 HCA compressor (single-shot window compression).

Computation per 128-token window:
    kv   = h @ w_kv            [128, 128]
    gate = h @ w_gate + bias   [128, 128]
    w    = softmax(gate, axis=tokens)   (per output channel)
    comp = sum(w * kv, axis=tokens)     [128]
then RMS-norm over channels and interleaved RoPE on the last 64 channels.

Sharding: 128 windows (2 batches x 64) split across 8 cores, 16 windows each.

v2 pipeline (per core, per window):
  - DMA h [128 tok, 2048] fp32 (SP ring; ~2.9us/window = the 360 GB/s floor)
  - GPSIMD (idle otherwise) converts fp32 -> bf16 in 4 chunks
  - PE transposes the bf16 chunks (bf16 streams at 1 cycle/row vs fp32's 2;
    the BIR verifier forbids mixing 32-bit and 16-bit matmul operands, so the
    conversion pass is what buys the fast transpose)
  - ACT/DVE copy PSUM->SBUF to build hT (bf16 TensorCopy hits the DVE 2x mode)
  - bf16 matmuls (1 cycle/row at any moving size) accumulate kv/gate per window
  - position bias added into the gate PSUM via an identity matmul
  - ACT exp with fused per-window sum; DVE mul+reduce for the numerator
Per group of 4 windows: transpose comp columns into rows, RMS-norm via a
Newton rsqrt on DVE (no ACT table switches), RoPE, and an early 2KB output
DMA so the tail only carries the last group's epilogue.
"""

import sys

if "/opt/trn_rl_repo" not in sys.path:
    sys.path.insert(0, "/opt/trn_rl_repo")

import ml_dtypes
import numpy as np

import concourse.bacc as bacc
import concourse.mybir as mybir
import concourse.tile as tile
from concourse.bass_utils import run_bass_kernel_spmd
from concourse.masks import make_identity

# Problem shapes (hardcoded per contest contract)
B, S, H = 2, 8192, 2048
M = 128          # compress rate (window length)
D = 128          # head dim
T = S // M       # 64 windows per batch
NCORES = 8
WPC = (B * T) // NCORES   # 16 windows per core
GW = 4                    # windows per output group
GROUPS = WPC // GW        # 4
KC = H // 128             # 16 contraction chunks
KB = KC // 4              # 4 chunk-banks (4 chunks share a PSUM bank)
ROPE_DIM = 64
HALF = ROPE_DIM // 2
THETA = 10000.0
EPS = 1e-6

F32 = mybir.dt.float32
BF16 = mybir.dt.bfloat16
I32 = mybir.dt.int32
AF = mybir.ActivationFunctionType
ALU = mybir.AluOpType


def _build_nc():
    nc = bacc.Bacc(None, target_bir_lowering=False)

    h_in = nc.dram_tensor("h_in", [WPC * M, H], F32, kind="ExternalInput")
    # weights host-preshuffled to [128, KC*D] so the DMA is a straight
    # 4KB-per-partition copy (keeps descriptors >= 512B at bf16)
    wkv_in = nc.dram_tensor("wkv_in", [128, KC * D], BF16, kind="ExternalInput")
    wg_in = nc.dram_tensor("wg_in", [128, KC * D], BF16, kind="ExternalInput")
    bias_in = nc.dram_tensor("bias_in", [D, GW * M], BF16, kind="ExternalInput")
    cs_in = nc.dram_tensor("cs_in", [GW, GROUPS * 2 * ROPE_DIM], F32, kind="ExternalInput")
    wn_in = nc.dram_tensor("wn_in", [GW, D], F32, kind="ExternalInput")
    out_d = nc.dram_tensor("out_d", [WPC, D], F32, kind="ExternalOutput")

    with tile.TileContext(nc) as tc:
        with (
            tc.tile_pool(name="constp", bufs=1) as constp,
            tc.tile_pool(name="hnatp", bufs=6) as hnatp,
            tc.tile_pool(name="hbp", bufs=3) as hbp,
            tc.tile_pool(name="hTp", bufs=2) as hTp,
            tc.tile_pool(name="sfxp", bufs=2) as sfxp,
            tc.tile_pool(name="smallp", bufs=2) as smallp,
            tc.tile_pool(name="grpp", bufs=2) as grpp,
            tc.tile_pool(name="tpp", bufs=3, space="PSUM") as tpp,
            tc.tile_pool(name="mmp", bufs=2, space="PSUM") as mmp,
            tc.tile_pool(name="ctp", bufs=1, space="PSUM") as ctp,
            tc.tile_pool(name="finalp", bufs=1) as finalp,
        ):
            # --- constants ---
            identb = constp.tile([128, 128], BF16, name="identb")
            make_identity(nc, identb)
            identf = constp.tile([128, 128], F32, name="identf")
            make_identity(nc, identf)
            zc = constp.tile([128, 1], F32, name="zc")
            nc.vector.memset(zc[:, :], 0.0)
            # preload the exp ACT table while the first DMAs run
            warm = constp.tile([128, 1], F32, name="warm")
            nc.scalar.activation(warm[:, :], zc[:, :], AF.Exp, bias=zc[:, :])

            # small constants ride the ACT ring so the SP ring streams h
            cs_sb = constp.tile([GW, GROUPS * 2 * ROPE_DIM], F32, name="cs_sb")
            nc.scalar.dma_start(out=cs_sb, in_=cs_in[:, :])
            wn_sb = constp.tile([GW, D], F32, name="wn_sb")
            nc.scalar.dma_start(out=wn_sb, in_=wn_in[:, :])

            wkv_sb = constp.tile([128, KC * D], BF16, name="wkv_sb")
            wg_sb = constp.tile([128, KC * D], BF16, name="wg_sb")
            bias_sb = constp.tile([D, GW * M], BF16, name="bias_sb")

            comp = finalp.tile([D, WPC], F32, name="comp")

            def emit_epilogue(g):
                """Group epilogue: transpose comp columns to rows, RMS-norm
                via Pool-engine Newton rsqrt (keeps DVE/ACT queues free for
                the steady-state pipeline), RoPE, early 2KB output DMA."""
                ct4 = ctp.tile([GW, D], F32, name="ct4", tag="ct4")
                nc.tensor.transpose(
                    ct4[:, :], comp[:, g * GW : (g + 1) * GW], identf[:, :]
                )
                sqs = grpp.tile([GW, D], F32, name="sqs", tag="sqs")
                ssq = grpp.tile([GW, 1], F32, name="ssq", tag="ssq")
                nc.scalar.activation(
                    sqs[:, :], ct4[:, :], AF.Square, bias=zc[:GW, :],
                    accum_out=ssq[:, :],
                )
                # norm weight + RoPE on unscaled rows (rinv is per-row, so it
                # commutes with the elementwise rope); DVE does this while
                # Pool runs the rsqrt chain
                t0 = grpp.tile([GW, D], F32, name="t0", tag="t0")
                nc.vector.tensor_mul(t0[:, :], ct4[:, :], wn_sb[:, :])
                t1 = grpp.tile([GW, ROPE_DIM], F32, name="t1", tag="t1")
                t2 = grpp.tile([GW, ROPE_DIM], F32, name="t2", tag="t2")
                c0 = g * 2 * ROPE_DIM
                nc.vector.tensor_mul(
                    t1[:, :], t0[:, D - ROPE_DIM : D], cs_sb[:, c0 : c0 + ROPE_DIM]
                )
                nc.vector.tensor_mul(
                    t2[:, 0:HALF],
                    t0[:, D - HALF : D],
                    cs_sb[:, c0 + ROPE_DIM : c0 + ROPE_DIM + HALF],
                )
                nc.vector.tensor_mul(
                    t2[:, HALF:ROPE_DIM],
                    t0[:, D - ROPE_DIM : D - HALF],
                    cs_sb[:, c0 + ROPE_DIM + HALF : c0 + 2 * ROPE_DIM],
                )
                nc.vector.tensor_add(t0[:, D - ROPE_DIM : D], t1[:, :], t2[:, :])
                # rinv = 1/sqrt(ssq/D + eps): bit-trick + 1 Newton step
                # (~1.7e-3 rel err; no ACT table switches)
                vv = grpp.tile([GW, 1], F32, name="vv", tag="vv")
                nc.vector.tensor_scalar(
                    out=vv[:, :], in0=ssq[:, :], scalar1=1.0 / D, scalar2=EPS,
                    op0=ALU.mult, op1=ALU.add,
                )
                rinv = grpp.tile([GW, 1], F32, name="rinv", tag="rinv")
                nc.vector.tensor_scalar(
                    out=rinv.bitcast(I32), in0=vv.bitcast(I32),
                    scalar1=1, scalar2=-1,
                    op0=ALU.arith_shift_right, op1=ALU.bitwise_xor,
                )
                nc.vector.tensor_scalar(
                    out=rinv.bitcast(I32), in0=rinv.bitcast(I32),
                    scalar1=0x5F3759DF + 1, scalar2=None, op0=ALU.add,
                )
                nt = grpp.tile([GW, 1], F32, name="nt", tag="nt")
                nc.vector.tensor_mul(nt[:, :], rinv[:, :], rinv[:, :])
                nc.vector.tensor_mul(nt[:, :], nt[:, :], vv[:, :])
                nc.vector.tensor_scalar(
                    out=nt[:, :], in0=nt[:, :], scalar1=-0.5, scalar2=1.5,
                    op0=ALU.mult, op1=ALU.add,
                )
                nc.vector.tensor_mul(rinv[:, :], rinv[:, :], nt[:, :])
                outg = grpp.tile([GW, D], F32, name="outg", tag="outg")
                nc.vector.tensor_scalar_mul(outg[:, :], t0[:, :], rinv[:, :])
                return outg

            def emit_out_dma(g, outg):
                nc.sync.dma_start(
                    out=out_d[g * GW : (g + 1) * GW, :], in_=outg[:, :]
                )

            pending_epi = None
            pending_out = None
            for w in range(WPC):
                last_w = w == WPC - 1
                hnat = hnatp.tile([128, H], F32, name="hnat", tag="hnat")
                tok0 = w * M
                if last_w:
                    # finer chunks for the last window to shorten the tail
                    for kb in range(KB):
                        nc.sync.dma_start(
                            out=hnat[:, kb * 512 : (kb + 1) * 512],
                            in_=h_in[tok0 : tok0 + M, kb * 512 : (kb + 1) * 512],
                        )
                else:
                    nc.sync.dma_start(out=hnat, in_=h_in[tok0 : tok0 + M, :])
                if w == 0:
                    # weights ride behind window 0 on the SP ring: PE idles
                    # early on the first matmuls but recovers; the h stream
                    # itself never stalls
                    nc.sync.dma_start(out=wkv_sb, in_=wkv_in[:, :])
                    nc.sync.dma_start(out=wg_sb, in_=wg_in[:, :])
                    nc.sync.dma_start(out=bias_sb, in_=bias_in[:, :])
                # output DMA 3 windows after its group: the data is long
                # ready, so this never head-blocks the SP h-stream queue
                if pending_out is not None and w % GW == 3:
                    emit_out_dma(*pending_out)
                    pending_out = None

                hb = hbp.tile([128, H], BF16, name="hb", tag="hb")
                hT = hTp.tile([128, KC * M], BF16, name="hT", tag="hT")
                for kb in range(KB):
                    src = hnat[:, kb * 512 : (kb + 1) * 512]
                    tp = tpp.tile([128, 512], F32 if last_w else BF16,
                                  name="tp", tag="tp")
                    if last_w:
                        # last window: fp32 transposes straight from hnat
                        # (2 cycles/row but drops the conversion stage from
                        # the tail's serial chain)
                        for j in range(4):
                            k = kb * 4 + j
                            nc.tensor.matmul(
                                tp[:, j * M : (j + 1) * M],
                                hnat[:, k * 128 : (k + 1) * 128],
                                identf[:, :],
                                is_transpose=True,
                                start=(j == 0),
                                stop=(j == 3),
                            )
                    else:
                        # fp32->bf16 conversion: GPSIMD (0.6 efficiency,
                        # 806ns/chunk) takes 3 chunks, ACT the 4th; bf16
                        # transposes then stream at 1 cycle/row vs fp32's 2
                        cdst = hb[:, kb * 512 : (kb + 1) * 512]
                        if kb < 3:
                            nc.gpsimd.tensor_copy(cdst, src)
                        else:
                            nc.scalar.copy(cdst, src)
                        for j in range(4):
                            k = kb * 4 + j
                            nc.tensor.matmul(
                                tp[:, j * M : (j + 1) * M],
                                hb[:, k * 128 : (k + 1) * 128],
                                identb[:, :],
                                is_transpose=True,
                                start=(j == 0),
                                stop=(j == 3),
                            )
                    dst = hT[:, kb * 512 : (kb + 1) * 512]
                    # bf16 TensorCopy hits the DVE 2x mode - DVE takes 3,
                    # ACT takes 1
                    if kb < 3:
                        nc.vector.tensor_copy(dst, tp[:, :])
                    else:
                        nc.scalar.copy(dst, tp[:, :])

                # one-window-delayed epilogue emission: its small ops queue
                # BEHIND this window's copies so they never head-block the
                # steady-state pipeline
                if pending_epi is not None:
                    pending_out = (pending_epi, emit_epilogue(pending_epi))
                    pending_epi = None

                # per-window projections: bf16 runs 1 cycle/row at moving=128.
                # gate and kv need separate PSUM tiles (interleaved
                # accumulation groups in one tile corrupt the first group).
                gt_ps = mmp.tile([D, M], F32, name="gt_ps", tag="gt")
                kv_ps = mmp.tile([D, M], F32, name="kv_ps", tag="kv")
                for kb in range(KB):
                    for j in range(4):
                        k = kb * 4 + j
                        nc.tensor.matmul(
                            gt_ps[:, :],
                            wg_sb[:, k * D : (k + 1) * D],
                            hT[:, k * M : (k + 1) * M],
                            start=(k == 0),
                            stop=False,
                        )
                    for j in range(4):
                        k = kb * 4 + j
                        nc.tensor.matmul(
                            kv_ps[:, :],
                            wkv_sb[:, k * D : (k + 1) * D],
                            hT[:, k * M : (k + 1) * M],
                            start=(k == 0),
                            stop=(k == KC - 1),
                        )
                # gate += position_bias via identity matmul (broadcast-free)
                nc.tensor.matmul(
                    gt_ps[:, :], identb[:, :], bias_sb[:, 0:M],
                    start=False, stop=True,
                )

                # softmax-weighted reduction over the window's tokens
                e_sb = sfxp.tile([D, M], F32, name="e_sb", tag="e")
                prod = sfxp.tile([D, M], F32, name="prod", tag="prod")
                den = smallp.tile([D, 1], F32, name="den", tag="den")
                num = smallp.tile([D, 1], F32, name="num", tag="num")
                rden = smallp.tile([D, 1], F32, name="rden", tag="rden")
                nc.scalar.activation(
                    e_sb[:, :], gt_ps[:, :], AF.Exp, bias=zc[:D, :],
                    accum_out=den[:, :],
                )
                nc.vector.tensor_mul(prod[:, :], e_sb[:, :], kv_ps[:, :])
                nc.vector.tensor_reduce(
                    num[:, :], prod[:, :], axis=mybir.AxisListType.X, op=ALU.add
                )
                nc.vector.reciprocal(rden[:, :], den[:, :])
                nc.vector.tensor_mul(comp[:, w : w + 1], num[:, :], rden[:, :])

                if w % GW == GW - 1:
                    if last_w:
                        emit_out_dma(w // GW, emit_epilogue(w // GW))
                    else:
                        pending_epi = w // GW

    nc.compile()
    return nc


_NC_CACHE = {}


def _get_nc():
    if "nc" not in _NC_CACHE:
        _NC_CACHE["nc"] = _build_nc()
    return _NC_CACHE["nc"]


def _make_in_maps(hidden_states, w_kv, w_gate, position_bias, kv_norm_weight):
    hidden_states = np.ascontiguousarray(np.asarray(hidden_states, dtype=np.float32))
    w_kv = np.asarray(w_kv, dtype=np.float32)
    w_gate = np.asarray(w_gate, dtype=np.float32)
    position_bias = np.asarray(position_bias, dtype=np.float32)
    kv_norm_weight = np.asarray(kv_norm_weight, dtype=np.float32)

    h_flat = hidden_states.reshape(B * S, H)
    # [H, D] -> [128, KC*D] so chunk k sits at cols [k*D, (k+1)*D)
    def shuffle_w(w):
        return np.ascontiguousarray(
            w.reshape(KC, 128, D).transpose(1, 0, 2).reshape(128, KC * D)
        ).astype(ml_dtypes.bfloat16)

    wkv_h = shuffle_w(w_kv)
    wg_h = shuffle_w(w_gate)
    bias_h = np.ascontiguousarray(np.tile(position_bias.T, (1, GW))).astype(
        ml_dtypes.bfloat16
    )
    wn = np.ascontiguousarray(np.broadcast_to(kv_norm_weight[None, :], (GW, D)))

    inv_freq = (1.0 / (THETA ** (np.arange(HALF, dtype=np.float32) / HALF))).astype(
        np.float32
    )
    in_maps = []
    for c in range(NCORES):
        t_global = (c % (T // WPC)) * WPC + np.arange(WPC, dtype=np.float32)
        pos = (t_global * M).astype(np.float32)
        freqs = pos[:, None] * inv_freq[None, :]
        cos2 = np.repeat(np.cos(freqs), 2, axis=1).astype(np.float32)
        sin2 = np.repeat(np.sin(freqs), 2, axis=1).astype(np.float32)
        sinf = np.concatenate([-sin2[:, :HALF], sin2[:, HALF:]], axis=1)
        cs16 = np.concatenate([cos2, sinf], axis=1)  # [16, 128]
        # window w = 4g+i -> cs[i, g*128 : (g+1)*128]
        cs = np.zeros((GW, GROUPS * 2 * ROPE_DIM), np.float32)
        for g in range(GROUPS):
            for i in range(GW):
                cs[i, g * 128 : (g + 1) * 128] = cs16[g * GW + i]
        cs = np.ascontiguousarray(cs)
        in_maps.append(
            {
                "h_in": h_flat[c * WPC * M : (c + 1) * WPC * M],
                "wkv_in": wkv_h,
                "wg_in": wg_h,
                "bias_in": bias_h,
                "cs_in": cs,
                "wn_in": wn,
            }
        )
    return in_maps


def _assemble(results):
    full = np.concatenate([r["out_d"] for r in results], axis=0)  # [128, 128]
    return full.reshape(B, 1, T, D).astype(np.float32)


def _run(inputs, trace=False, **spmd_kwargs):
    nc = _get_nc()
    in_maps = _make_in_maps(
        inputs["hidden_states"],
        inputs["w_kv"],
        inputs["w_gate"],
        inputs["position_bias"],
        inputs["kv_norm_weight"],
    )
    res = run_bass_kernel_spmd(
        nc, in_maps, core_ids=list(range(NCORES)), trace=trace, **spmd_kwargs
    )
    return _assemble(res.results), res


def kernel(
    hidden_states,
    q_residual=None,
    position_ids=None,
    w_kv=None,
    w_gate=None,
    position_bias=None,
    kv_norm_weight=None,
):
    out, _ = _run(
        {
            "hidden_states": hidden_states,
            "w_kv": w_kv,
            "w_gate": w_gate,
            "position_bias": position_bias,
            "kv_norm_weight": kv_norm_weight,
        }
    )
    return out
